# revision 1
# baseline (speedup 1.0000x reference)
"""Deformable bilinear sampling kernel for TRN2 (8-core SPMD).

Algorithm: per (n,o) pair, each output pixel (h,w) needs the 2x2x32c patch at
(h+floor(off_h), w+floor(off_w)) with bilinear corner weights. The host stages
a patch-replicated DRAM tensor P[pair] where row (hh*144+ww) holds the
contiguous 128-float patch at padded anchor (hh,ww); the device computes int16
gather indices + corner weights from the offsets, pulls one 512B row per pixel
with gpsimd.dma_gather, and does a 4-term weighted combine on DVE/GPSIMD.
"""

import numpy as np

import concourse.bacc as bacc
import concourse.bass as bass
import concourse.mybir as mybir
from concourse.library_config import mlp

PAIRS = 4          # (n,o) pairs per core
H = W = 128
C = 32
PAD = 8
HP = 144           # padded anchor grid
NROWS = HP * HP    # 20736 patch rows per pair
NIDX = H * W       # 16384 gathered pixels per pair
CH = 2             # gather chunks per pair
NIDX_CH = NIDX // CH
WCH = W // CH      # w-columns per chunk
NCHUNK = PAIRS * CH

F32 = mybir.dt.float32
I16 = mybir.dt.int16
OP = mybir.AluOpType
TWO23 = 12582912.0  # 1.5 * 2^23: forces round-to-integer in f32 for |x| < 2^22


def build_nc(combine_split=None):
    """combine_split: list of 'v'(vector) or 'g'(gpsimd) per chunk (len 8)."""
    if combine_split is None:
        combine_split = ["v"] * NCHUNK
        combine_split[6] = "g"
        combine_split[7] = "g"
    nc = bacc.Bacc("TRN2")
    patches = nc.declare_dram_parameter("patches", [PAIRS, NROWS, 128], F32, isOutput=False)
    offn = nc.declare_dram_parameter("offn", [PAIRS, 2, H, W], F32, isOutput=False)
    basen = nc.declare_dram_parameter("basen", [H, W], F32, isOutput=False)
    out = nc.declare_dram_parameter("out", [PAIRS, H, W, C], F32, isOutput=True)

    from contextlib import ExitStack

    with ExitStack() as stack:
        ec = stack.enter_context
        block = ec(nc.Block())
        NG = 4   # gather buffers
        NA = 4   # acc buffers
        Gb = [ec(nc.sbuf_tensor(f"G{i}", [128, NIDX_CH // 128, 128], F32)) for i in range(NG)]
        accb = [ec(nc.sbuf_tensor(f"acc{i}", [128, WCH, C], F32)) for i in range(NA)]
        tmpv = ec(nc.sbuf_tensor("tmpv", [128, WCH, C], F32))
        tmpg = ec(nc.sbuf_tensor("tmpg", [128, WCH, C], F32))
        on0 = ec(nc.sbuf_tensor("on0", [128, 2, W], F32))
        on1 = ec(nc.sbuf_tensor("on1", [128, 2, W], F32))
        bnat = ec(nc.sbuf_tensor("bnat", [128, W], F32))
        d0 = ec(nc.sbuf_tensor("d0", [128, 1024], I16))
        d1 = ec(nc.sbuf_tensor("d1", [128, 1024], I16))
        wt0 = ec(nc.sbuf_tensor("wt0", [128, 4, W], F32))
        wt1 = ec(nc.sbuf_tensor("wt1", [128, 4, W], F32))
        sf = ec(nc.sbuf_tensor("sf", [128, 2, W], F32))      # frac (natural)
        sg = ec(nc.sbuf_tensor("sg", [128, 2, W], F32))      # 1-frac (natural)
        sy2 = ec(nc.sbuf_tensor("sy2", [128, 2, W], F32))
        tD = ec(nc.sbuf_tensor("tD", [128, W], F32))
        dnat = ec(nc.sbuf_tensor("dnat", [128, W], I16))
        s_inb = ec(nc.semaphore("s_inb"))    # basew DMA
        s_in0 = ec(nc.semaphore("s_in0"))    # input DMAs for buffer set 0
        s_in1 = ec(nc.semaphore("s_in1"))    # input DMAs for buffer set 1
        s_g = [ec(nc.semaphore(f"s_g{i}")) for i in range(NCHUNK)]  # gather i done
        s_cmb = [ec(nc.semaphore(f"s_cmb{i}")) for i in range(NCHUNK)]  # combine i done
        s_out = [ec(nc.semaphore(f"s_out{i}")) for i in range(NCHUNK)]  # out i done
        s_inx = [s_in0, s_in1]
        s_dn = ec(nc.semaphore("s_dn"))      # dnat ready (inc 1/pair)
        s_wt = ec(nc.semaphore("s_wt"))      # weights ready (inc 1/pair)
        s_dw = ec(nc.semaphore("s_dw"))      # ACT wrap copies (inc 1 each, 16/pair)
        s_cv = ec(nc.semaphore("s_cv"))      # vector same-engine chain
        s_cg = ec(nc.semaphore("s_cg"))      # gpsimd same-engine chain
        onb = [on0, on1]
        db = [d0, d1]
        wtb = [wt0, wt1]


        @block.sync
        def _(sync: bass.BassEngine):
            sync.dma_start(bnat[:, :], basen[:, :]).then_inc(s_inb, 16)
            for p in range(min(2, PAIRS)):
                sync.dma_start(onb[p % 2][:, :, :], offn[p, :, :, :].transpose([1, 0, 2])).then_inc(s_inx[p % 2], 16)
            for s in range(NCHUNK):
                p, c = divmod(s, CH)
                if c == 0 and p + 2 < PAIRS:
                    # refill input tile of set p%2: vector must be done with
                    # pair p's idx/weights math (it reads on[st]).
                    sync.wait_ge(s_dn, p + 1)
                    pp = p + 2
                    sync.dma_start(onb[pp % 2][:, :, :], offn[pp, :, :, :].transpose([1, 0, 2])).then_inc(s_inx[pp % 2], 16)
                # out DMA for chunk s
                sync.wait_ge(s_cmb[s], 1)
                dst = out[p, :, c * WCH:(c + 1) * WCH, :]   # (h, w, c)
                sync.dma_start(dst, accb[s % NA][:, :, :]).then_inc(s_out[s], 16)

        class Chain:
            """Serializes dependent ops on one engine via a chain semaphore:
            wait for all previously-registered ops, then run the thunk and
            register its instruction."""

            def __init__(self, eng, sem):
                self.eng, self.sem, self.n = eng, sem, 0
                self.extra = []

            def run(self, thunk, final=None):
                # final=(sem, value_after): inc that sem instead of the chain
                if self.n:
                    self.eng.wait_ge(self.sem, self.n)
                for sem, val in self.extra:
                    self.eng.wait_ge(sem, val)
                self.extra = []
                inst = thunk()
                if final is None:
                    inst.then_inc(self.sem, 1)
                    self.n += 1
                else:
                    sem, val = final
                    inst.then_inc(sem, 1)
                    self.extra.append((sem, val))
                return inst

        def idx_weights(eng, ch, p):
            st = p % 2
            onf = onb[st][:, :, :]      # [128, 2, W] natural offsets
            r = ch.run
            wt = wtb[st]
            # ---- floors/fracs (natural layout, shared) ----
            r(lambda: eng.tensor_scalar(sy2[:, :, :], onf, TWO23, -TWO23, OP.add, OP.add))
            r(lambda: eng.tensor_tensor(sf[:, :, :], sy2[:, :, :], onf, OP.is_gt))
            r(lambda: eng.tensor_sub(sy2[:, :, :], sy2[:, :, :], sf[:, :, :]))   # floors
            r(lambda: eng.tensor_sub(sf[:, :, :], onf, sy2[:, :, :]))            # frac
            # ---- gather indices first (unblocks ACT + the gather DMA asap) ----
            r(lambda: eng.scalar_tensor_tensor(tD[:, :], sy2[:, 0, :], float(HP), sy2[:, 1, :], OP.mult, OP.add))
            r(lambda: eng.tensor_add(tD[:, :], tD[:, :], bnat[:, :]))
            r(lambda: eng.tensor_copy(dnat[:, :], tD[:, :]), final=(s_dn, p + 1))
            # ---- weights ----
            r(lambda: eng.tensor_scalar(sg[:, :, :], sf[:, :, :], -1.0, 1.0, OP.mult, OP.add))
            r(lambda: eng.tensor_mul(wt[:, 0, :], sg[:, 0, :], sg[:, 1, :]))
            r(lambda: eng.tensor_mul(wt[:, 1, :], sg[:, 0, :], sf[:, 1, :]))
            r(lambda: eng.tensor_mul(wt[:, 2, :], sf[:, 0, :], sg[:, 1, :]))
            r(lambda: eng.tensor_mul(wt[:, 3, :], sf[:, 0, :], sf[:, 1, :]), final=(s_wt, p + 1))
            return ch

        def emit_combine(eng, ch, s, tmp):
            p, c = divmod(s, CH)
            st = p % 2
            G = Gb[s % NG]
            acc = accb[s % NA]
            gflat = G[:, :, :]  # [128, WCH, 128]; slot k = cols k*C:(k+1)*C
            wt = wtb[st]
            ws = c * WCH
            r = ch.run

            def gk(k):
                return gflat[:, :, k * C:(k + 1) * C]

            def wk(k):
                a = wt[:, k, ws:ws + WCH]          # [128, WCH]
                return a.unsqueeze(2).broadcast_to([128, WCH, C])

            r(lambda: eng.tensor_mul(acc[:, :, :], gk(0), wk(0)))
            r(lambda: eng.tensor_mul(tmp[:, :, :], gk(1), wk(1)))
            r(lambda: eng.tensor_add(acc[:, :, :], acc[:, :, :], tmp[:, :, :]))
            r(lambda: eng.tensor_mul(tmp[:, :, :], gk(2), wk(2)))
            r(lambda: eng.tensor_add(acc[:, :, :], acc[:, :, :], tmp[:, :, :]))
            r(lambda: eng.tensor_mul(tmp[:, :, :], gk(3), wk(3)))
            return lambda final: r(
                lambda: eng.tensor_add(acc[:, :, :], acc[:, :, :], tmp[:, :, :]),
                final=final,
            )

        @block.vector
        def _(vector: bass.BassEngine):
            ch = Chain(vector, s_cv)
            # one-time: zero the wrapped-idx tiles (the gather AP spans all 128
            # partitions; only 0-31 carry real data)
            ch.run(lambda: vector.memset(d0[:, :], 0))
            ch.run(lambda: vector.memset(d1[:, :], 0))
            vector.wait_ge(s_inb, 16)
            for p in range(min(2, PAIRS)):
                vector.wait_ge(s_inx[p % 2], 16)
                if p >= 1:
                    # dnat reuse: ACT wrap-copies of pair p-1 must be done
                    vector.wait_ge(s_dw, 84 * p - 16)
                idx_weights(vector, ch, p)
            for s in range(NCHUNK):
                p, c = divmod(s, CH)
                if combine_split[s] == "v":
                    vector.wait_ge(s_g[s], 16)
                    if s >= NA:
                        vector.wait_ge(s_out[s - NA], 16)
                    emit_combine(vector, ch, s, tmpv)((s_cmb[s], 1))
                if c == CH - 1 and p + 2 < PAIRS:
                    pp = p + 2
                    vector.wait_ge(s_inx[pp % 2], 16 * (pp // 2 + 1))
                    # dnat reuse: ACT wrap-copies of pair pp-1 must be done
                    vector.wait_ge(s_dw, 84 * pp - 16)
                    # wt[p%2] reuse: combines of pair p must be done
                    vector.wait_ge(s_cmb[CH * p], 1)
                    vector.wait_ge(s_cmb[CH * p + 1], 1)
                    idx_weights(vector, ch, pp)

        @block.scalar
        def _(act: bass.BassEngine):
            # rearrange dnat [128h, 128w] -> wrapped d[st] partitions 0-31:
            # d[g*16+q, w*8+k] = dnat[q+16k, w]  (g = replication group)
            for p in range(PAIRS):
                st = p % 2
                if p >= 1:
                    act.wait_ge(s_dw, 84 * p)   # drain own prior-pair DMA incs
                act.wait_ge(s_dn, p + 1)
                if p >= 2:
                    # d[st] reuse: gathers of pair p-2 must be done
                    act.wait_ge(s_g[CH * (p - 2)], 16)
                    act.wait_ge(s_g[CH * (p - 2) + 1], 16)
                dwrap = db[st][:, :].rearrange("p (w k) -> p w k", k=8)
                for k in range(0, 8, 2):   # even k: engine copy (32-aligned src)
                    act.copy(dwrap[0:16, :, k],
                             dnat[16 * k:16 * (k + 1), :]).then_inc(s_dw, 1)
                with nc.allow_non_contiguous_dma(reason="4KB idx-wrap strided dst"):
                    for k in range(1, 8, 2):   # odd k: tiny DMA (no partition align)
                        act.dma_start(dwrap[0:16, :, k],
                                      dnat[16 * k:16 * (k + 1), :]).then_inc(s_dw, 16)
                # engine copies + DMAs above: 4*1 + 4*16 = 68 incs per pair
                act.wait_ge(s_dw, 84 * p + 68)
                # replicate wrapped indices to partitions 16-31 (the group the
                # Q7 descriptor-gen core actually reads on HW)
                act.dma_start(db[st][16:32, :], db[st][0:16, :]).then_inc(s_dw, 16)

        @block.gpsimd
        def _(gpsimd: bass.BassGpSimd):
            chg = Chain(gpsimd, s_cg)
            gpsimd.load_library(mlp)
            for s in range(NCHUNK):
                p, c = divmod(s, CH)
                gpsimd.wait_ge(s_dw, 84 * (p + 1))
                if s >= NG:
                    gpsimd.wait_ge(s_cmb[s - NG], 1)  # G[s%NG] free
                gpsimd.dma_gather(
                    Gb[s % NG][:, :, :],
                    patches[p, :, :],
                    db[p % 2][:, c * 512:(c + 1) * 512],
                    NIDX_CH,
                    NIDX_CH,
                    128,
                    single_packet=False,
                ).then_inc(s_g[s], 16)
            for s in range(NCHUNK):
                if combine_split[s] == "g":
                    gpsimd.wait_ge(s_wt, s // CH + 1)
                    gpsimd.wait_ge(s_g[s], 16)
                    if s >= NA:
                        gpsimd.wait_ge(s_out[s - NA], 16)
                    emit_combine(gpsimd, chg, s, tmpg)((s_cmb[s], 1))

    nc.compile()
    return nc


# ---------------- host-side helpers ----------------

def build_patches_all(imgs_pairs):
    """imgs_pairs: (NPAIR, C, H, W) f32 -> (NPAIR, NROWS, 128) f32"""
    npair = imgs_pairs.shape[0]
    hw_c = np.ascontiguousarray(np.transpose(imgs_pairs, (0, 2, 3, 1)))  # (P,H,W,C)
    padded = np.zeros((npair, HP + 1, HP + 1, C), np.float32)
    padded[:, PAD:PAD + H, PAD:PAD + W] = hw_c
    P = np.empty((npair, HP, HP, 4, C), np.float32)
    P[:, :, :, 0] = padded[:, 0:HP, 0:HP]
    P[:, :, :, 1] = padded[:, 0:HP, 1:HP + 1]
    P[:, :, :, 2] = padded[:, 1:HP + 1, 0:HP]
    P[:, :, :, 3] = padded[:, 1:HP + 1, 1:HP + 1]
    return P.reshape(npair, NROWS, 128)


def base_natural():
    h = np.arange(H).reshape(H, 1)
    w = np.arange(W).reshape(1, W)
    return ((h + PAD) * HP + (w + PAD)).astype(np.float32)


def make_in_map(imgs_pairs, offp):
    return {
        "patches": build_patches_all(imgs_pairs),
        "offn": np.ascontiguousarray(offp),
        "basen": base_natural(),
    }


# ---------------- public entry point ----------------

N_CORES = 8
PAIRS_TOTAL = 32

LAST_EXEC_TIME_NS = None


def kernel(images, offsets):
    """images (4,8,32,128,128) f32; offsets (4,16,128,128) f32 ->
    (4,8,32,128,128) f32 deformable bilinear sampling, on 8 NeuronCores."""
    import os
    global LAST_EXEC_TIME_NS
    from concourse.bass_utils import run_bass_kernel_spmd

    images = np.ascontiguousarray(np.asarray(images, dtype=np.float32))
    offsets = np.ascontiguousarray(np.asarray(offsets, dtype=np.float32))
    imgs = images.reshape(PAIRS_TOTAL, C, H, W)
    offp = offsets.reshape(4, 8, 2, H, W).reshape(PAIRS_TOTAL, 2, H, W)

    nc = build_nc()
    in_maps = []
    for core in range(N_CORES):
        sl = slice(core * PAIRS, (core + 1) * PAIRS)
        in_maps.append(make_in_map(imgs[sl], offp[sl]))
    trace = bool(os.environ.get("DK_TRACE"))
    res = run_bass_kernel_spmd(nc, in_maps, list(range(N_CORES)), trace=trace)
    if trace:
        LAST_EXEC_TIME_NS = res.exec_time_ns
        if res.instructions_and_trace:
            print("trace path:", res.instructions_and_trace[1])
    outs = [np.asarray(res.results[i]["out"]) for i in range(N_CORES)]
    full = np.concatenate(outs, axis=0)            # (32, H, W, C)
    full = np.transpose(full, (0, 3, 1, 2))        # (32, C, H, W)
    return np.ascontiguousarray(full.reshape(4, 8, C, H, W)).astype(np.float32)

